# revision 17
# baseline (speedup 1.0000x reference)
"""Mixture-of-Depths Trainium2 kernel (8 NeuronCores, token-sharded).

Math per token row x (d=2048):
    h   = gelu(x @ Wc1 + bc1)                  # [512]
    lg  = h @ Wc2 + bc2                        # scalar logit
    cont= sigmoid(lg) < 0.8  <=>  lg < ln(4)   # (monotone)
    y   = gelu(x @ Wf1 + bf1) @ Wf2 + bf2      # [2048]
    out = cont ? y : x

Sharding: 16384 tokens split 8 ways (2048/core); weights replicated.
Per core, tokens are processed in 4 blocks of 512. The FFN matmuls
(97% of FLOPs) run in fp16 (1 cycle/row, 2-byte LDWEIGHTS hides under
the 512-col stream); the confidence head runs in float32r (~tf32
numerics) so the routing mask keeps extra precision — a flipped mask
near the 0.8 threshold would inject an O(1) error. PSUM accumulation
is fp32 everywhere. X is pre-transposed host-side so the contraction
dim lands on SBUF partitions; weights are re-tiled host-side so every
DMA is a large contiguous read. bf2 and the confidence threshold are
applied on the vector engine (fp32), not in PSUM.

Measured on 8 axon-tunneled TRN2 NeuronCores: HW exec ~1.89 ms
(PE ~95% busy; matmul cadence 216 ns = the N=512 streaming floor),
relative error vs the fp32 jax reference ~4.2e-4.
"""

import sys

for _p in ("/opt/trn_rl_repo",):
    if _p not in sys.path:
        sys.path.append(_p)

import numpy as np

import concourse.bass as bass
import concourse.tile as tile
import concourse.mybir as mybir
from concourse.vector_clock import ScopedClock
from concourse.bass_utils import run_bass_kernel_spmd

P = 128
NCORES = 8
D_MODEL = 2048
D_CONF = 512
D_FF = 8192
T_CORE = 2048          # tokens per core
TB = 512               # token block
NTB = T_CORE // TB     # 4
TS = TB // P           # 4 token sub-blocks per block
KD = D_MODEL // P      # 16
KC = D_CONF // P       # 4
NF = D_FF // P         # 64
ND = D_MODEL // 512    # 4
THRESHOLD_LOGIT = float(np.log(np.float64(0.8) / np.float64(0.2)))  # ln 4

MM_DT = mybir.dt.float32r   # matmul operand dtype


class SplitDrainTileContext(tile.TileContext):
    """This walrus build supports at most ONE sync wait per instruction.
    Hoist extra waits onto same-engine nofuse nops at commit time, and
    re-emit the final Drain's waits as individual nops."""

    def _commit_instruction(self, inst, lazy_reg_writes: bool = True):
        si = getattr(inst, "sync_info", None)
        if (
            si is not None
            and si.on_wait
            and len(si.on_wait) > 1
            and inst.engine != mybir.EngineType.Unassigned
        ):
            waits = list(si.on_wait)
            inst.sync_info = mybir.SyncInfo(
                on_wait=[waits[-1]], on_update=list(si.on_update or [])
            )
            for w in waits[:-1]:
                nop = mybir.InstNoOp(
                    name=self.nc.get_next_instruction_name(),
                    sync_info=mybir.SyncInfo(on_wait=[w], on_update=[]),
                    bass_nofuse=True,
                    engine=inst.engine,
                )
                super()._commit_instruction(nop, lazy_reg_writes=False)
        super()._commit_instruction(inst, lazy_reg_writes)

    def _drain_and_barrier(self, tick_clock, wait_clock):
        nc = self.nc
        probe = nc.sync.nop()
        wait_clock.add_sem_waits(
            probe.ins, ScopedClock({None: tick_clock.global_clock})
        )
        waits = list(probe.ins.sync_info.on_wait) if probe.ins.sync_info else []
        probe.ins.sync_info = None
        for w in waits:
            n = nc.sync.nop()
            n.ins.sync_info = mybir.SyncInfo(on_wait=[w], on_update=[])
        nc.sync.drain()
        nc.all_engine_barrier()
        assert self.sems is not None
        popped = nc._tile_sem_poison_stack.pop()
        assert popped is self._sem_poison
        nc.clear_and_free_semaphores(list(self.sems.allocated().values()))
        nc.all_engine_barrier()


def build_program():
    f32 = mybir.dt.float32
    fr = MM_DT
    fh = mybir.dt.float16
    nc = bass.Bass()

    xt = nc.dram_tensor("xt", [NTB, P, KD, TB], fr, kind="ExternalInput")
    xt16 = nc.dram_tensor("xt16", [NTB, P, KD, TB], fh, kind="ExternalInput")
    xn = nc.dram_tensor("xn", [NTB * TS, ND, P, 512], f32, kind="ExternalInput")
    wc1 = nc.dram_tensor("wc1", [KC, P, KD, P], fr, kind="ExternalInput")
    wc2 = nc.dram_tensor("wc2", [KC, P, 8], fr, kind="ExternalInput")
    bc1 = nc.dram_tensor("bc1", [KC, P], f32, kind="ExternalInput")
    wf1 = nc.dram_tensor("wf1", [NF, P, KD, P], fh, kind="ExternalInput")
    bf1 = nc.dram_tensor("bf1", [NF, P], f32, kind="ExternalInput")
    wf2 = nc.dram_tensor("wf2", [ND, NF, P, 512], fh, kind="ExternalInput")
    # bf2 broadcast across partitions, [p, d, j] = bf2[d*512+j]
    bf2b = nc.dram_tensor("bf2b", [P, ND, 512], f32, kind="ExternalInput")
    # per-partition copy of (bc2 - threshold); mask = (logit + nthrb < 0)
    nthrb = nc.dram_tensor("nthrb", [P, 1], f32, kind="ExternalInput")
    out = nc.dram_tensor("out", [NTB * TS, ND, P, 512], f32, kind="ExternalOutput")

    gelu = mybir.ActivationFunctionType.Gelu
    add_op = mybir.AluOpType.add
    is_lt = mybir.AluOpType.is_lt
    is_ge = mybir.AluOpType.is_ge

    with SplitDrainTileContext(nc) as tc:
        with (
            tc.tile_pool(name="consts", bufs=1) as consts,
            tc.tile_pool(name="xpool", bufs=1) as xpool,
            tc.tile_pool(name="gpool", bufs=1) as gpool,
            tc.tile_pool(name="w1cpool", bufs=3) as w1cpool,
            tc.tile_pool(name="w1fpool", bufs=4) as w1fpool,
            tc.tile_pool(name="w2pool", bufs=12) as w2pool,
            tc.tile_pool(name="xnpool", bufs=3) as xnpool,
            tc.tile_pool(name="opool", bufs=3) as opool,
            tc.tile_pool(name="psg", bufs=3, space="PSUM") as psg,
            tc.tile_pool(name="psy", bufs=4, space="PSUM") as psy,
            tc.tile_pool(name="psl", bufs=1, space="PSUM") as psl,
            tc.tile_pool(name="hpool", bufs=2) as hpool,
        ):
            # ---- constants resident for the whole kernel ----
            wc2_s = consts.tile([P, KC, 8], fr)
            nc.sync.dma_start(out=wc2_s[:], in_=wc2.ap().rearrange("c p r -> p c r"))
            bc1_s = consts.tile([P, KC], f32)
            nc.sync.dma_start(out=bc1_s[:], in_=bc1.ap().rearrange("c p -> p c"))
            bf1_s = consts.tile([P, NF], f32)
            nc.sync.dma_start(out=bf1_s[:], in_=bf1.ap().rearrange("f p -> p f"))
            nthr_s = consts.tile([P, 1], f32)
            nc.sync.dma_start(out=nthr_s[:], in_=nthrb.ap())
            bf2_s = consts.tile([P, ND, 512], f32)
            nc.sync.dma_start(out=bf2_s[:], in_=bf2b.ap())
            # per-token select masks, one column per token sub-block
            mask_s = consts.tile([P, NTB * TS], f32)
            nmask_s = consts.tile([P, NTB * TS], f32)

            for tb in range(NTB):
                xt_s = xpool.tile([P, KD, TB], fr)         # 32KB/part
                xt16_s = xpool.tile([P, KD, TB], fh)       # 16KB/part
                for kq in range(0, KD, 4):
                    nc.sync.dma_start(
                        out=xt16_s[:, kq : kq + 4, :],
                        in_=xt16.ap()[tb, :, kq : kq + 4, :],
                    )
                    nc.sync.dma_start(
                        out=xt_s[:, kq : kq + 4, :],
                        in_=xt.ap()[tb, :, kq : kq + 4, :],
                    )

                g = gpool.tile([P, NF, TB], fh)            # 64KB/part
                h1 = hpool.tile([P, KC, TB], fr)           # 8KB/part


                # ---- FFN mm1: g = gelu(x @ Wf1 + bf1), transposed layout ----
                for f in range(NF):
                    wt = w1fpool.tile([P, KD, P], fh, tag="w1f")
                    nc.sync.dma_start(out=wt[:], in_=wf1.ap()[f])
                    pg = psg.tile([P, TB], f32, tag="ps")
                    for k in range(KD):
                        nc.tensor.matmul(
                            out=pg[:],
                            lhsT=wt[:, k, :],
                            rhs=xt16_s[:, k, :],
                            start=(k == 0),
                            stop=(k == KD - 1),
                        )
                    nc.scalar.activation(
                        out=g[:, f, :], in_=pg[:], func=gelu,
                        bias=bf1_s[:, f : f + 1], scale=1.0,
                    )

                # ---- confidence head ----
                for c in range(KC):
                    wt = w1cpool.tile([P, KD, P], fr, tag="w1c")
                    nc.sync.dma_start(out=wt[:], in_=wc1.ap()[c])
                    ps = psg.tile([P, TB], f32, tag="ps")
                    for k in range(KD):
                        nc.tensor.matmul(
                            out=ps[:],
                            lhsT=wt[:, k, :],
                            rhs=xt_s[:, k, :],
                            start=(k == 0),
                            stop=(k == KD - 1),
                        )
                    nc.scalar.activation(
                        out=h1[:, c, :], in_=ps[:], func=gelu,
                        bias=bc1_s[:, c : c + 1], scale=1.0,
                    )
                for ts in range(TS):
                    col = tb * TS + ts
                    pl = psl.tile([P, 8], f32)
                    for c in range(KC):
                        nc.tensor.matmul(
                            out=pl[:],
                            lhsT=h1[:, c, ts * P : (ts + 1) * P],
                            rhs=wc2_s[:, c, :],
                            start=(c == 0),
                            stop=(c == KC - 1),
                        )
                    # mask = (logit + (bc2 - thr)) < 0 ; nmask = complement
                    nc.vector.tensor_scalar(
                        mask_s[:, col : col + 1], pl[:, 0:1],
                        nthr_s[:, 0:1], 0.0, add_op, is_lt,
                    )
                    nc.vector.tensor_scalar(
                        nmask_s[:, col : col + 1], pl[:, 0:1],
                        nthr_s[:, 0:1], 0.0, add_op, is_ge,
                    )

                # ---- FFN mm2 + bias + select + store ----
                for d in range(ND):
                    pys = []
                    for ts in range(TS):
                        pys.append(psy.tile([P, 512], f32, tag="py", name=f"py_{d}_{ts}"))
                    for f in range(NF):
                        w2 = w2pool.tile([P, 512], fh)
                        nc.sync.dma_start(out=w2[:], in_=wf2.ap()[d, f])
                        for ts in range(TS):
                            nc.tensor.matmul(
                                out=pys[ts][:],
                                lhsT=g[:, f, ts * P : (ts + 1) * P],
                                rhs=w2[:],
                                start=(f == 0),
                                stop=(f == NF - 1),
                            )
                    for ts in range(TS):
                        row = tb * TS + ts
                        xn_t = xnpool.tile([P, 512], f32)
                        nc.sync.dma_start(out=xn_t[:], in_=xn.ap()[row, d])
                        o = opool.tile([P, 512], f32)
                        # o = (y + bf2) * mask + x * (1 - mask)
                        nc.vector.tensor_add(
                            out=o[:], in0=pys[ts][:], in1=bf2_s[:, d, :]
                        )
                        nc.vector.tensor_scalar_mul(
                            o[:], o[:], mask_s[:, row : row + 1]
                        )
                        nc.vector.tensor_scalar_mul(
                            xn_t[:], xn_t[:], nmask_s[:, row : row + 1]
                        )
                        nc.vector.tensor_add(out=o[:], in0=o[:], in1=xn_t[:])
                        nc.sync.dma_start(out=out.ap()[row, d], in_=o[:])
    return nc


_NC_CACHE = None


def _get_program():
    global _NC_CACHE
    if _NC_CACHE is None:
        _NC_CACHE = build_program()
    return _NC_CACHE


def _prepare_in_maps(inputs):
    hs = np.asarray(inputs["hidden_states"], dtype=np.float32)
    Wc1 = np.asarray(inputs["Wc1"], dtype=np.float32)
    bc1 = np.asarray(inputs["bc1"], dtype=np.float32)
    Wc2 = np.asarray(inputs["Wc2"], dtype=np.float32)
    bc2 = np.asarray(inputs["bc2"], dtype=np.float32)
    Wf1 = np.asarray(inputs["Wf1"], dtype=np.float32)
    bf1 = np.asarray(inputs["bf1"], dtype=np.float32)
    Wf2 = np.asarray(inputs["Wf2"], dtype=np.float32)
    bf2 = np.asarray(inputs["bf2"], dtype=np.float32)
    layer_idx = int(np.asarray(inputs["layer_idx"]))

    B, S, D = hs.shape
    assert D == D_MODEL and B * S == NCORES * T_CORE

    X = np.ascontiguousarray(hs.reshape(B * S, D))

    # ---- shared (replicated) weight layouts ----
    wc1_r = np.ascontiguousarray(
        Wc1.reshape(KD, P, KC, P).transpose(2, 1, 0, 3))
    wc2_r = np.ascontiguousarray(
        np.broadcast_to(Wc2.reshape(KC, P, 1), (KC, P, 8)))
    bc1_r = np.ascontiguousarray(bc1.reshape(KC, P))
    wf1_r = np.ascontiguousarray(
        Wf1.reshape(KD, P, NF, P).transpose(2, 1, 0, 3).astype(np.float16))
    bf1_r = np.ascontiguousarray(bf1.reshape(NF, P))
    wf2_r = np.ascontiguousarray(
        Wf2.reshape(NF, P, ND, 512).transpose(2, 0, 1, 3).astype(np.float16))
    bf2b_r = np.ascontiguousarray(
        np.broadcast_to(bf2.reshape(1, ND, 512), (P, ND, 512)))
    if layer_idx < 1:      # MIN_LAYERS=1: every token continues
        # mask = (logit + nthr < 0) must be TRUE everywhere
        nthr_v = np.float32(-1e30)
    else:
        nthr_v = np.float32(float(bc2.reshape(-1)[0]) - THRESHOLD_LOGIT)
    nthrb_r = np.full((P, 1), nthr_v, np.float32)

    shared = dict(wc1=wc1_r, wc2=wc2_r, bc1=bc1_r, wf1=wf1_r, bf1=bf1_r,
                  wf2=wf2_r, bf2b=bf2b_r, nthrb=nthrb_r)

    in_maps = []
    for c in range(NCORES):
        Xc = X[c * T_CORE : (c + 1) * T_CORE]
        # xt[tb,p,k,t] = Xc[tb*TB+t, k*128+p]
        xt_c = np.ascontiguousarray(
            Xc.reshape(NTB, TB, KD, P).transpose(0, 3, 2, 1))
        xt16_c = xt_c.astype(np.float16)
        # xn[row,d,p,j] = Xc[row*128+p, d*512+j]
        xn_c = np.ascontiguousarray(
            Xc.reshape(NTB * TS, P, ND, 512).transpose(0, 2, 1, 3))
        in_maps.append(dict(shared, xt=xt_c, xt16=xt16_c, xn=xn_c))
    return in_maps, (B, S, D)


def _gather_output(res, shape):
    B, S, D = shape
    out = np.empty((NCORES * T_CORE, D_MODEL), np.float32)
    for c in range(NCORES):
        oc = res.results[c]["out"]            # [16, ND, P, 512]
        out[c * T_CORE : (c + 1) * T_CORE] = (
            oc.transpose(0, 2, 1, 3).reshape(T_CORE, D_MODEL))
    return out.reshape(B, S, D)


def kernel(**inputs) -> np.ndarray:
    in_maps, shape = _prepare_in_maps(inputs)
    nc = _get_program()
    res = run_bass_kernel_spmd(nc, in_maps, list(range(NCORES)))
    return _gather_output(res, shape)


def run_traced(inputs, **kw):
    """Run with NTFF profiling; returns BassKernelResults (test harness use)."""
    in_maps, shape = _prepare_in_maps(inputs)
    nc = _get_program()
    res = run_bass_kernel_spmd(nc, in_maps, list(range(NCORES)), trace=True, **kw)
    res.output = _gather_output(res, shape)
    return res


# revision 18
# speedup vs baseline: 1.0023x; 1.0023x over previous
"""Mixture-of-Depths Trainium2 kernel (8 NeuronCores, token-sharded).

Math per token row x (d=2048):
    h   = gelu(x @ Wc1 + bc1)                  # [512]
    lg  = h @ Wc2 + bc2                        # scalar logit
    cont= sigmoid(lg) < 0.8  <=>  lg < ln(4)   # (monotone)
    y   = gelu(x @ Wf1 + bf1) @ Wf2 + bf2      # [2048]
    out = cont ? y : x

Sharding: 16384 tokens split 8 ways (2048/core); weights replicated.
Per core, tokens are processed in 4 blocks of 512. The FFN matmuls
(97% of FLOPs) run in fp16 (1 cycle/row, 2-byte LDWEIGHTS hides under
the 512-col stream); the confidence head runs in float32r (~tf32
numerics) so the routing mask keeps extra precision — a flipped mask
near the 0.8 threshold would inject an O(1) error. PSUM accumulation
is fp32 everywhere. X is pre-transposed host-side so the contraction
dim lands on SBUF partitions; weights are re-tiled host-side so every
DMA is a large contiguous read. bf2 and the confidence threshold are
applied on the vector engine (fp32), not in PSUM.

Measured on 8 axon-tunneled TRN2 NeuronCores: HW exec ~1.89 ms
(PE ~95% busy; matmul cadence 216 ns = the N=512 streaming floor),
relative error vs the fp32 jax reference ~4.2e-4.
"""

import sys

for _p in ("/opt/trn_rl_repo",):
    if _p not in sys.path:
        sys.path.append(_p)

import numpy as np

import concourse.bass as bass
import concourse.tile as tile
import concourse.mybir as mybir
from concourse.vector_clock import ScopedClock
from concourse.bass_utils import run_bass_kernel_spmd

P = 128
NCORES = 8
D_MODEL = 2048
D_CONF = 512
D_FF = 8192
T_CORE = 2048          # tokens per core
TB = 512               # token block
NTB = T_CORE // TB     # 4
TS = TB // P           # 4 token sub-blocks per block
KD = D_MODEL // P      # 16
KC = D_CONF // P       # 4
NF = D_FF // P         # 64
ND = D_MODEL // 512    # 4
THRESHOLD_LOGIT = float(np.log(np.float64(0.8) / np.float64(0.2)))  # ln 4

MM_DT = mybir.dt.float32r   # matmul operand dtype


class SplitDrainTileContext(tile.TileContext):
    """This walrus build supports at most ONE sync wait per instruction.
    Hoist extra waits onto same-engine nofuse nops at commit time, and
    re-emit the final Drain's waits as individual nops."""

    def _commit_instruction(self, inst, lazy_reg_writes: bool = True):
        si = getattr(inst, "sync_info", None)
        if (
            si is not None
            and si.on_wait
            and len(si.on_wait) > 1
            and inst.engine != mybir.EngineType.Unassigned
        ):
            waits = list(si.on_wait)
            inst.sync_info = mybir.SyncInfo(
                on_wait=[waits[-1]], on_update=list(si.on_update or [])
            )
            for w in waits[:-1]:
                nop = mybir.InstNoOp(
                    name=self.nc.get_next_instruction_name(),
                    sync_info=mybir.SyncInfo(on_wait=[w], on_update=[]),
                    bass_nofuse=True,
                    engine=inst.engine,
                )
                super()._commit_instruction(nop, lazy_reg_writes=False)
        super()._commit_instruction(inst, lazy_reg_writes)

    def _drain_and_barrier(self, tick_clock, wait_clock):
        nc = self.nc
        probe = nc.sync.nop()
        wait_clock.add_sem_waits(
            probe.ins, ScopedClock({None: tick_clock.global_clock})
        )
        waits = list(probe.ins.sync_info.on_wait) if probe.ins.sync_info else []
        probe.ins.sync_info = None
        for w in waits:
            n = nc.sync.nop()
            n.ins.sync_info = mybir.SyncInfo(on_wait=[w], on_update=[])
        nc.sync.drain()
        nc.all_engine_barrier()
        assert self.sems is not None
        popped = nc._tile_sem_poison_stack.pop()
        assert popped is self._sem_poison
        nc.clear_and_free_semaphores(list(self.sems.allocated().values()))
        nc.all_engine_barrier()


def build_program():
    f32 = mybir.dt.float32
    fr = MM_DT
    fh = mybir.dt.float16
    nc = bass.Bass()

    xt = nc.dram_tensor("xt", [NTB, P, KD, TB], fr, kind="ExternalInput")
    xt16 = nc.dram_tensor("xt16", [NTB, P, KD, TB], fh, kind="ExternalInput")
    xn = nc.dram_tensor("xn", [NTB * TS, ND, P, 512], f32, kind="ExternalInput")
    wc1 = nc.dram_tensor("wc1", [KC, P, KD, P], fr, kind="ExternalInput")
    wc2 = nc.dram_tensor("wc2", [KC, P, 8], fr, kind="ExternalInput")
    bc1 = nc.dram_tensor("bc1", [KC, P], f32, kind="ExternalInput")
    wf1 = nc.dram_tensor("wf1", [NF, P, KD, P], fh, kind="ExternalInput")
    bf1 = nc.dram_tensor("bf1", [NF, P], f32, kind="ExternalInput")
    wf2 = nc.dram_tensor("wf2", [ND, NF, P, 512], fh, kind="ExternalInput")
    # bf2 broadcast across partitions, [p, d, j] = bf2[d*512+j]
    bf2b = nc.dram_tensor("bf2b", [P, ND, 512], f32, kind="ExternalInput")
    # per-partition copy of (bc2 - threshold); mask = (logit + nthrb < 0)
    nthrb = nc.dram_tensor("nthrb", [P, 1], f32, kind="ExternalInput")
    out = nc.dram_tensor("out", [NTB * TS, ND, P, 512], f32, kind="ExternalOutput")

    gelu = mybir.ActivationFunctionType.Gelu
    add_op = mybir.AluOpType.add
    is_lt = mybir.AluOpType.is_lt
    is_ge = mybir.AluOpType.is_ge

    with SplitDrainTileContext(nc) as tc:
        with (
            tc.tile_pool(name="consts", bufs=1) as consts,
            tc.tile_pool(name="xpool", bufs=1) as xpool,
            tc.tile_pool(name="gpool", bufs=1) as gpool,
            tc.tile_pool(name="w1cpool", bufs=3) as w1cpool,
            tc.tile_pool(name="w1fpool", bufs=4) as w1fpool,
            tc.tile_pool(name="w2pool", bufs=12) as w2pool,
            tc.tile_pool(name="xnpool", bufs=3) as xnpool,
            tc.tile_pool(name="opool", bufs=3) as opool,
            tc.tile_pool(name="psg", bufs=2, space="PSUM") as psg,
            tc.tile_pool(name="psy", bufs=6, space="PSUM") as psy,
            tc.tile_pool(name="hpool", bufs=2) as hpool,
        ):
            # ---- constants resident for the whole kernel ----
            wc2_s = consts.tile([P, KC, 8], fr)
            nc.sync.dma_start(out=wc2_s[:], in_=wc2.ap().rearrange("c p r -> p c r"))
            bc1_s = consts.tile([P, KC], f32)
            nc.sync.dma_start(out=bc1_s[:], in_=bc1.ap().rearrange("c p -> p c"))
            bf1_s = consts.tile([P, NF], f32)
            nc.sync.dma_start(out=bf1_s[:], in_=bf1.ap().rearrange("f p -> p f"))
            nthr_s = consts.tile([P, 1], f32)
            nc.sync.dma_start(out=nthr_s[:], in_=nthrb.ap())
            bf2_s = consts.tile([P, ND, 512], f32)
            nc.sync.dma_start(out=bf2_s[:], in_=bf2b.ap())
            # per-token select masks, one column per token sub-block
            mask_s = consts.tile([P, NTB * TS], f32)
            nmask_s = consts.tile([P, NTB * TS], f32)

            for tb in range(NTB):
                xt_s = xpool.tile([P, KD, TB], fr)         # 32KB/part
                xt16_s = xpool.tile([P, KD, TB], fh)       # 16KB/part
                for kq in range(0, KD, 4):
                    nc.sync.dma_start(
                        out=xt16_s[:, kq : kq + 4, :],
                        in_=xt16.ap()[tb, :, kq : kq + 4, :],
                    )
                    nc.sync.dma_start(
                        out=xt_s[:, kq : kq + 4, :],
                        in_=xt.ap()[tb, :, kq : kq + 4, :],
                    )

                g = gpool.tile([P, NF, TB], fh)            # 64KB/part
                h1 = hpool.tile([P, KC, TB], fr)           # 8KB/part


                # ---- FFN mm1: g = gelu(x @ Wf1 + bf1), transposed layout ----
                for f in range(NF):
                    wt = w1fpool.tile([P, KD, P], fh, tag="w1f")
                    nc.sync.dma_start(out=wt[:], in_=wf1.ap()[f])
                    pg = psg.tile([P, TB], f32, tag="ps")
                    for k in range(KD):
                        nc.tensor.matmul(
                            out=pg[:],
                            lhsT=wt[:, k, :],
                            rhs=xt16_s[:, k, :],
                            start=(k == 0),
                            stop=(k == KD - 1),
                        )
                    nc.scalar.activation(
                        out=g[:, f, :], in_=pg[:], func=gelu,
                        bias=bf1_s[:, f : f + 1], scale=1.0,
                    )

                # ---- confidence head ----
                for c in range(KC):
                    wt = w1cpool.tile([P, KD, P], fr, tag="w1c")
                    nc.sync.dma_start(out=wt[:], in_=wc1.ap()[c])
                    ps = psg.tile([P, TB], f32, tag="ps")
                    for k in range(KD):
                        nc.tensor.matmul(
                            out=ps[:],
                            lhsT=wt[:, k, :],
                            rhs=xt_s[:, k, :],
                            start=(k == 0),
                            stop=(k == KD - 1),
                        )
                    nc.scalar.activation(
                        out=h1[:, c, :], in_=ps[:], func=gelu,
                        bias=bc1_s[:, c : c + 1], scale=1.0,
                    )
                for ts in range(TS):
                    col = tb * TS + ts
                    pl = psg.tile([P, 8], f32, tag="ps", name=f"pl_{tb}_{ts}")
                    for c in range(KC):
                        nc.tensor.matmul(
                            out=pl[:],
                            lhsT=h1[:, c, ts * P : (ts + 1) * P],
                            rhs=wc2_s[:, c, :],
                            start=(c == 0),
                            stop=(c == KC - 1),
                        )
                    # mask = (logit + (bc2 - thr)) < 0 ; nmask = complement
                    nc.vector.tensor_scalar(
                        mask_s[:, col : col + 1], pl[:, 0:1],
                        nthr_s[:, 0:1], 0.0, add_op, is_lt,
                    )
                    nc.vector.tensor_scalar(
                        nmask_s[:, col : col + 1], pl[:, 0:1],
                        nthr_s[:, 0:1], 0.0, add_op, is_ge,
                    )

                # ---- FFN mm2 + bias + select + store ----
                for d in range(ND):
                    pys = []
                    for ts in range(TS):
                        pys.append(psy.tile([P, 512], f32, tag="py", name=f"py_{d}_{ts}"))
                    for f in range(NF):
                        w2 = w2pool.tile([P, 512], fh)
                        nc.sync.dma_start(out=w2[:], in_=wf2.ap()[d, f])
                        for ts in range(TS):
                            nc.tensor.matmul(
                                out=pys[ts][:],
                                lhsT=g[:, f, ts * P : (ts + 1) * P],
                                rhs=w2[:],
                                start=(f == 0),
                                stop=(f == NF - 1),
                            )
                    for ts in range(TS):
                        row = tb * TS + ts
                        xn_t = xnpool.tile([P, 512], f32)
                        nc.sync.dma_start(out=xn_t[:], in_=xn.ap()[row, d])
                        o = opool.tile([P, 512], f32)
                        # o = (y + bf2) * mask + x * (1 - mask)
                        nc.vector.tensor_add(
                            out=o[:], in0=pys[ts][:], in1=bf2_s[:, d, :]
                        )
                        nc.vector.tensor_scalar_mul(
                            o[:], o[:], mask_s[:, row : row + 1]
                        )
                        nc.vector.tensor_scalar_mul(
                            xn_t[:], xn_t[:], nmask_s[:, row : row + 1]
                        )
                        nc.vector.tensor_add(out=o[:], in0=o[:], in1=xn_t[:])
                        nc.sync.dma_start(out=out.ap()[row, d], in_=o[:])
    return nc


_NC_CACHE = None


def _get_program():
    global _NC_CACHE
    if _NC_CACHE is None:
        _NC_CACHE = build_program()
    return _NC_CACHE


def _prepare_in_maps(inputs):
    hs = np.asarray(inputs["hidden_states"], dtype=np.float32)
    Wc1 = np.asarray(inputs["Wc1"], dtype=np.float32)
    bc1 = np.asarray(inputs["bc1"], dtype=np.float32)
    Wc2 = np.asarray(inputs["Wc2"], dtype=np.float32)
    bc2 = np.asarray(inputs["bc2"], dtype=np.float32)
    Wf1 = np.asarray(inputs["Wf1"], dtype=np.float32)
    bf1 = np.asarray(inputs["bf1"], dtype=np.float32)
    Wf2 = np.asarray(inputs["Wf2"], dtype=np.float32)
    bf2 = np.asarray(inputs["bf2"], dtype=np.float32)
    layer_idx = int(np.asarray(inputs["layer_idx"]))

    B, S, D = hs.shape
    assert D == D_MODEL and B * S == NCORES * T_CORE

    X = np.ascontiguousarray(hs.reshape(B * S, D))

    # ---- shared (replicated) weight layouts ----
    wc1_r = np.ascontiguousarray(
        Wc1.reshape(KD, P, KC, P).transpose(2, 1, 0, 3))
    wc2_r = np.ascontiguousarray(
        np.broadcast_to(Wc2.reshape(KC, P, 1), (KC, P, 8)))
    bc1_r = np.ascontiguousarray(bc1.reshape(KC, P))
    wf1_r = np.ascontiguousarray(
        Wf1.reshape(KD, P, NF, P).transpose(2, 1, 0, 3).astype(np.float16))
    bf1_r = np.ascontiguousarray(bf1.reshape(NF, P))
    wf2_r = np.ascontiguousarray(
        Wf2.reshape(NF, P, ND, 512).transpose(2, 0, 1, 3).astype(np.float16))
    bf2b_r = np.ascontiguousarray(
        np.broadcast_to(bf2.reshape(1, ND, 512), (P, ND, 512)))
    if layer_idx < 1:      # MIN_LAYERS=1: every token continues
        # mask = (logit + nthr < 0) must be TRUE everywhere
        nthr_v = np.float32(-1e30)
    else:
        nthr_v = np.float32(float(bc2.reshape(-1)[0]) - THRESHOLD_LOGIT)
    nthrb_r = np.full((P, 1), nthr_v, np.float32)

    shared = dict(wc1=wc1_r, wc2=wc2_r, bc1=bc1_r, wf1=wf1_r, bf1=bf1_r,
                  wf2=wf2_r, bf2b=bf2b_r, nthrb=nthrb_r)

    in_maps = []
    for c in range(NCORES):
        Xc = X[c * T_CORE : (c + 1) * T_CORE]
        # xt[tb,p,k,t] = Xc[tb*TB+t, k*128+p]
        xt_c = np.ascontiguousarray(
            Xc.reshape(NTB, TB, KD, P).transpose(0, 3, 2, 1))
        xt16_c = xt_c.astype(np.float16)
        # xn[row,d,p,j] = Xc[row*128+p, d*512+j]
        xn_c = np.ascontiguousarray(
            Xc.reshape(NTB * TS, P, ND, 512).transpose(0, 2, 1, 3))
        in_maps.append(dict(shared, xt=xt_c, xt16=xt16_c, xn=xn_c))
    return in_maps, (B, S, D)


def _gather_output(res, shape):
    B, S, D = shape
    out = np.empty((NCORES * T_CORE, D_MODEL), np.float32)
    for c in range(NCORES):
        oc = res.results[c]["out"]            # [16, ND, P, 512]
        out[c * T_CORE : (c + 1) * T_CORE] = (
            oc.transpose(0, 2, 1, 3).reshape(T_CORE, D_MODEL))
    return out.reshape(B, S, D)


def kernel(**inputs) -> np.ndarray:
    in_maps, shape = _prepare_in_maps(inputs)
    nc = _get_program()
    res = run_bass_kernel_spmd(nc, in_maps, list(range(NCORES)))
    return _gather_output(res, shape)


def run_traced(inputs, **kw):
    """Run with NTFF profiling; returns BassKernelResults (test harness use)."""
    in_maps, shape = _prepare_in_maps(inputs)
    nc = _get_program()
    res = run_bass_kernel_spmd(nc, in_maps, list(range(NCORES)), trace=True, **kw)
    res.output = _gather_output(res, shape)
    return res
